# revision 33
# baseline (speedup 1.0000x reference)
import os
import sys

sys.path.insert(0, "/opt/trn_rl_repo")

import numpy as np
import ml_dtypes
import bass_rust
from concourse import bass, mybir
from concourse.tile import TileContext
from concourse.vector_clock import ScopedClock
from concourse.bass_utils import run_bass_kernel_spmd

B, S, E, H = 4, 2048, 1024, 1024
NCORES = 8
NT = 8  # q-tiles per core, 128 rows each
EC = E // 128
HC = H // 128
KC = S // 128
F32 = mybir.dt.float32
BF16 = mybir.dt.bfloat16
BF16_NP = ml_dtypes.bfloat16

# Results of the last run_bass_kernel_spmd call (for test harness inspection).
LAST_RESULT = None


def _global_tile(core: int, t: int) -> int:
    """Local q-tile t on core -> global 128-row tile index r in [0, 16).

    Tiles are grouped in four causal classes (nb = t//2 + 1 key-blocks of
    512); each core takes two tiles per class so instruction streams are
    identical across cores.
    """
    half = core % 2
    return 4 * (t // 2) + 2 * half + (t % 2)


class PatchedTileContext(TileContext):
    """TileContext whose tail drain carries at most one sem wait.

    The walrus codegen in this container rejects a Drain with more than one
    sync wait ("Too many sync wait commands"); split the global-clock waits
    across a chain of drains on the same engine instead.
    """

    def _drain_and_barrier(self, tick_clock, wait_clock):
        drain_inst = self.nc.sync.drain()
        wait_clock.add_sem_waits(
            drain_inst.ins, ScopedClock({None: tick_clock.global_clock})
        )
        mi = drain_inst.ins
        waits = list(mi.sync_info.on_wait)
        ups = list(mi.sync_info.on_update)
        if len(waits) > 1:
            mi.sync_info = bass_rust.SyncInfo(on_wait=waits[:1], on_update=[])
            for i, w in enumerate(waits[1:]):
                d2 = self.nc.sync.drain()
                last = i == len(waits) - 2
                d2.ins.sync_info = bass_rust.SyncInfo(
                    on_wait=[w], on_update=ups if last else []
                )
        self.nc.all_engine_barrier()
        assert self.sems is not None
        popped = self.nc._tile_sem_poison_stack.pop()
        assert popped is self._sem_poison
        # Device-side semaphore/DMA-ring clears + the second barrier are only
        # needed if the NEFF executes again in the same NRT session; this
        # kernel runs once per launch, so skip ~8us of teardown. Host-side
        # bookkeeping (sem free-list) is irrelevant at the outermost tile.


def _split_multi_waits(json_bytes):
    """Rewrite BIR so no instruction carries more than one sync wait.

    The walrus build in this container rejects instructions with multiple
    sync waits ("Too many sync wait commands"). Engines execute in order, so
    hoisting the extra waits onto NoOp instructions inserted immediately
    before the original instruction is semantically equivalent.
    """
    import json as _json

    d = _json.loads(json_bytes)
    ctr = 0
    for f in d.get("functions", []):
        for blk in f.get("blocks", []):
            insts = blk.get("instructions", [])
            out = []
            for inst in insts:
                si = inst.get("sync_info") or {}
                ow = si.get("on_wait") or []
                if len(ow) > 1:
                    for w in ow[:-1]:
                        out.append(
                            {
                                "debug": inst.get("debug", 0),
                                "engine": inst["engine"],
                                "ins": [],
                                "name": f"wsplit_{ctr}",
                                "opcode": "NoOp",
                                "outs": [],
                                "sync_info": {"on_update": [], "on_wait": [w]},
                            }
                        )
                        ctr += 1
                    si = dict(si)
                    si["on_wait"] = [ow[-1]]
                    inst = dict(inst)
                    inst["sync_info"] = si
                out.append(inst)
            blk["instructions"] = out
    return _json.dumps(d).encode()


def _build_program():
    nc = bass.Bass("TRN2", target_bir_lowering=False, debug=False, num_devices=NCORES)
    orig_to_json_bytes = nc.to_json_bytes
    nc.to_json_bytes = lambda: _split_multi_waits(orig_to_json_bytes())

    # xT columns are host-permuted per core: within each 512-key block the
    # core's own two q-tiles come first, so q-tile t sits at fixed columns
    # (t//2)*512 + (t%2)*128 on EVERY core (uniform SPMD program).
    xT = nc.dram_tensor("xT", [E, S], BF16, kind="ExternalInput")
    wqT = nc.dram_tensor("wqT", [E, H], BF16, kind="ExternalInput")
    wkT = nc.dram_tensor("wkT", [E, H], BF16, kind="ExternalInput")
    wvT = nc.dram_tensor("wvT", [E, H], BF16, kind="ExternalInput")
    bqs = nc.dram_tensor("bqs", [H], F32, kind="ExternalInput")
    bk = nc.dram_tensor("bk", [H], F32, kind="ExternalInput")
    bv = nc.dram_tensor("bv", [H], F32, kind="ExternalInput")
    masks = nc.dram_tensor("masks", [NT, 128, 512], F32, kind="ExternalInput")
    ident = nc.dram_tensor("ident", [128, 128], BF16, kind="ExternalInput")
    out = nc.dram_tensor("out", [NT, 128, H], BF16, kind="ExternalOutput")

    with PatchedTileContext(nc) as tc:
        with (
            tc.tile_pool(name="const", bufs=1) as const_pool,
            tc.tile_pool(name="xp", bufs=1) as x_pool,
            tc.tile_pool(name="ktp", bufs=1) as kt_pool,
            tc.tile_pool(name="vp", bufs=1) as v_pool,
            tc.tile_pool(name="qtp", bufs=1) as qt_pool,
        ):
            cst = const_pool.tile([128, H + 2 * HC], F32, tag="cst")
            bv_bc = cst[:, 0:H]
            bq_t = cst[:, H : H + HC]
            bk_t = cst[:, H + HC : H + 2 * HC]
            idc = const_pool.tile([128, 128], BF16, tag="idc")

            # resident tensors (bf16): x for all three projections, K^T and V
            # for attention, Q^T per-tile slices
            x = x_pool.tile([128, EC, S], BF16, tag="x")
            kt = [
                kt_pool.tile([128, S], BF16, tag=f"kt{c}", name=f"kt{c}")
                for c in range(HC)
            ]
            v = [
                v_pool.tile([128, H], BF16, tag=f"v{k}", name=f"v{k}")
                for k in range(KC)
            ]
            # [128, hc, q-col]: q-tile t occupies columns t*128..t*128+128,
            # so each Q-projection drain writes a contiguous 512-col run
            qt = qt_pool.tile([128, HC, NT * 128], BF16, tag="qt")

            # the scores PSUM pool opens BEFORE everything else so the first
            # attention matmuls have no pool-handoff wait (LIFO: closed last)
            psum_outer = tc.tile_pool(name="sps", bufs=2, space="PSUM")
            sps_pool = psum_outer.__enter__()

            with (
                tc.tile_pool(name="wA", bufs=1) as wA_pool,
                tc.tile_pool(name="wB", bufs=1) as wB_pool,
            ):
                wk_sb = [
                    wA_pool.tile([128, H], BF16, tag=f"wA{e}", name=f"wk{e}")
                    for e in range(EC)
                ]
                wv_sb = [
                    wB_pool.tile([128, H], BF16, tag=f"wB{e}", name=f"wv{e}")
                    for e in range(EC)
                ]
                # interleave the first x columns with wk so the e=0 matmul can
                # issue early; stream the rest of x, then prefetch wv
                # spread the startup streams across four queue engines
                # (~140GB/s per queue) in consumption order so kb0 is never
                # starved; wq/wv carry slot-reuse waits so they must stay on
                # gpsimd (a waiting DMA would block a compute engine's queue)
                # stripe the startup streams round-robin across the three
                # DMA queue engines in consumption order: x-kb0 + first wk
                # half feed the split kb0 passes, then the rest of wk/x
                dma_engs = [nc.sync, nc.scalar, nc.gpsimd]
                rr = [0]

                def stripe(out_ap, in_ap):
                    dma_engs[rr[0] % 3].dma_start(out=out_ap, in_=in_ap)
                    rr[0] += 1

                for e in range(EC):
                    stripe(x[:, e, 0:512], xT[e * 128 : (e + 1) * 128, 0:512])
                    stripe(wk_sb[e][:, 0:512], wkT[e * 128 : (e + 1) * 128, 0:512])
                for e in range(EC):
                    stripe(wk_sb[e][:, 512:H], wkT[e * 128 : (e + 1) * 128, 512:H])
                for e in range(EC):
                    stripe(x[:, e, 512:1024], xT[e * 128 : (e + 1) * 128, 512:1024])
                nc.gpsimd.dma_start(
                    out=bq_t, in_=bqs[:].rearrange("(c p) -> p c", p=128)
                )
                nc.gpsimd.dma_start(
                    out=bk_t, in_=bk[:].rearrange("(c p) -> p c", p=128)
                )
                for e in range(EC):
                    xq_eng = nc.sync if e % 2 == 0 else nc.scalar
                    xq_eng.dma_start(
                        out=x[:, e, 1024:S], in_=xT[e * 128 : (e + 1) * 128, 1024:S]
                    )
                nc.gpsimd.dma_start(out=bv_bc, in_=bv[:].partition_broadcast(128))
                nc.sync.dma_start(out=idc, in_=ident[:, :])

                # projection pool shrinks to 6 banks (max 4 concurrent tiles)
                with tc.tile_pool(name="pp", bufs=6, space="PSUM") as pp_pool:
                    # ---- K projection: kt[hc] = (Wk x)^T + bk ----
                    # half-hc passes: the startup working set is 2MB
                    # (x-kb0 + wk half) instead of 3MB
                    for kb in range(4):
                        hc_groups = [[0, 1, 2, 3], [4, 5, 6, 7]]
                        for hcg in hc_groups:
                            ps = [
                                pp_pool.tile(
                                    [128, 512], F32, tag="pp", name=f"psk{kb}_{hc}"
                                )
                                for hc in hcg
                            ]
                            for e in range(EC):
                                for i, hc in enumerate(hcg):
                                    nc.tensor.matmul(
                                        ps[i],
                                        lhsT=wk_sb[e][:, hc * 128 : (hc + 1) * 128],
                                        rhs=x[:, e, kb * 512 : (kb + 1) * 512],
                                        start=(e == 0),
                                        stop=(e == EC - 1),
                                    )
                            for i, hc in enumerate(hcg):
                                nc.vector.tensor_scalar_add(
                                    kt[hc][:, kb * 512 : (kb + 1) * 512],
                                    ps[i],
                                    bk_t[:, hc : hc + 1],
                                )

                    # wq reuses the wk slots (tag reuse -> anti-dependency;
                    # with e-outer kb3 the per-e waits release early)
                    wq_sb = [
                        wA_pool.tile([128, H], BF16, tag=f"wA{e}", name=f"wq{e}")
                        for e in range(EC)
                    ]
                    for e in range(EC):
                        nc.gpsimd.dma_start(
                            out=wq_sb[e], in_=wqT[e * 128 : (e + 1) * 128, :]
                        )
                    for e in range(EC):
                        nc.gpsimd.dma_start(
                            out=wv_sb[e], in_=wvT[e * 128 : (e + 1) * 128, :]
                        )

                    # ---- Q projection (before V so the attention phase's
                    # PSUM handoff waits on V's cheap drains, not Q's):
                    # reads the core's q columns straight out of resident x
                    # (fixed offsets thanks to the host block permutation) ----
                    for qs in range(2):
                        for hcg in ([0, 1, 2, 3], [4, 5, 6, 7]):
                            ps = [
                                pp_pool.tile(
                                    [128, 512], F32, tag="pp", name=f"psq{qs}_{hc}"
                                )
                                for hc in hcg
                            ]
                            for e in range(EC):
                                xr = x[:, e, :].rearrange(
                                    "p (a b c) -> p a b c", a=4, b=4, c=128
                                )
                                rhs = xr[:, 2 * qs : 2 * qs + 2, 0:2, :]
                                for i, hc in enumerate(hcg):
                                    nc.tensor.matmul(
                                        ps[i],
                                        lhsT=wq_sb[e][:, hc * 128 : (hc + 1) * 128],
                                        rhs=rhs,
                                        start=(e == 0),
                                        stop=(e == EC - 1),
                                    )
                            for i, hc in enumerate(hcg):
                                nc.vector.tensor_scalar_add(
                                    qt[:, hc, qs * 512 : (qs + 1) * 512],
                                    ps[i],
                                    bq_t[:, hc : hc + 1],
                                )

                    # ---- V projection: v[kc] = x_kc^T Wv + bv ----
                    for kc in range(KC):
                        ps2 = [
                            pp_pool.tile(
                                [128, 512], F32, tag="pp", name=f"psv{kc}_{hh}"
                            )
                            for hh in range(2)
                        ]
                        for e in range(EC):
                            for hh in range(2):
                                nc.tensor.matmul(
                                    ps2[hh],
                                    lhsT=x[:, e, kc * 128 : (kc + 1) * 128],
                                    rhs=wv_sb[e][:, hh * 512 : (hh + 1) * 512],
                                    start=(e == 0),
                                    stop=(e == EC - 1),
                                )
                        for hh in range(2):
                            nc.vector.tensor_add(
                                v[kc][:, hh * 512 : (hh + 1) * 512],
                                ps2[hh],
                                bv_bc[:, hh * 512 : (hh + 1) * 512],
                            )

            # ---- attention, classes descending, 1-deep software pipeline:
            # scores(t_next) overlaps softmax+PV(t_prev) ----
            with (
                tc.tile_pool(name="mskp", bufs=2) as msk_pool,
                tc.tile_pool(name="ssbp", bufs=2) as ssb_pool,
                tc.tile_pool(name="ptp", bufs=4) as pt_pool,
                tc.tile_pool(name="outp", bufs=2) as out_pool,
                tc.tile_pool(name="stat", bufs=4) as stat_pool,
                tc.tile_pool(name="ops", bufs=4, space="PSUM") as ops_pool,
                tc.tile_pool(name="tps", bufs=2, space="PSUM") as tps_pool,
            ):

                def emit_scores(t):
                    nb = t // 2 + 1
                    msk = msk_pool.tile([128, 512], F32, tag="msk", name=f"msk{t}")
                    nc.sync.dma_start(out=msk, in_=masks[t, :, :])
                    ssb = ssb_pool.tile([128, 4, 512], BF16, tag="ssb", name=f"ssb{t}")
                    mparts = stat_pool.tile([128, 4], F32, tag="mparts", name=f"mp{t}")
                    for kb in range(nb):
                        sp = sps_pool.tile([128, 512], F32, tag="sp", name=f"sp{t}_{kb}")
                        for hc in range(HC):
                            nc.tensor.matmul(
                                sp,
                                lhsT=qt[:, hc, t * 128 : (t + 1) * 128],
                                rhs=kt[hc][:, kb * 512 : (kb + 1) * 512],
                                start=(hc == 0),
                                stop=(hc == HC - 1),
                            )
                        if kb == nb - 1:
                            nc.vector.tensor_add(ssb[:, kb, :], sp, msk)
                        else:
                            nc.vector.tensor_copy(ssb[:, kb, :], sp)
                        nc.vector.reduce_max(
                            mparts[:, kb : kb + 1],
                            ssb[:, kb, :],
                            axis=mybir.AxisListType.X,
                        )
                    nm = stat_pool.tile([128, 1], F32, tag="nm", name=f"nm{t}")
                    nc.vector.reduce_max(
                        nm, mparts[:, :nb], axis=mybir.AxisListType.X, negate=True
                    )
                    lparts = stat_pool.tile([128, 4], F32, tag="lparts", name=f"lp{t}")
                    for kb in range(nb):
                        nc.scalar.activation(
                            ssb[:, kb, :],
                            ssb[:, kb, :],
                            mybir.ActivationFunctionType.Exp,
                            bias=nm,
                            accum_out=lparts[:, kb : kb + 1],
                        )
                    return (t, nb, ssb, lparts)

                def emit_pv(ctx):
                    t, nb, ssb, lparts = ctx
                    nkc = nb * 4
                    # for even tiles the own-odd key chunk of the final block
                    # is entirely causally masked (P exactly 0) -> skip it
                    skip_kc = 4 * (nb - 1) + 1 if t % 2 == 0 else -1
                    kcs = [kc for kc in range(nkc) if kc != skip_kc]
                    lsum = stat_pool.tile([128, 1], F32, tag="lsum", name=f"ls{t}")
                    nc.vector.reduce_sum(
                        lsum, lparts[:, :nb], axis=mybir.AxisListType.X
                    )
                    rl = stat_pool.tile([128, 1], F32, tag="rl", name=f"rl{t}")
                    nc.vector.reciprocal(rl, lsum)
                    po = [
                        ops_pool.tile([128, 512], F32, tag="po", name=f"po{t}_{hh}")
                        for hh in range(2)
                    ]
                    pts = {}

                    def do_T(kc):
                        tp = tps_pool.tile(
                            [128, 128], BF16, tag="tp", name=f"tp{t}_{kc}"
                        )
                        nc.tensor.transpose(
                            tp,
                            ssb[:, kc // 4, (kc % 4) * 128 : (kc % 4 + 1) * 128],
                            idc,
                        )
                        pt = pt_pool.tile([128, 128], BF16, tag="pt", name=f"pt{t}_{kc}")
                        nc.vector.tensor_copy(pt, tp)
                        pts[kc] = pt

                    for j in range(min(2, len(kcs))):
                        do_T(kcs[j])
                    for j, kc in enumerate(kcs):
                        if j + 2 < len(kcs):
                            do_T(kcs[j + 2])
                        for hh in range(2):
                            nc.tensor.matmul(
                                po[hh],
                                lhsT=pts[kc],
                                rhs=v[kc][:, hh * 512 : (hh + 1) * 512],
                                start=(j == 0),
                                stop=(j == len(kcs) - 1),
                            )
                    # normalize in quarters on two engines (scalar first so
                    # nothing queues ahead of the next tile's pt copies on
                    # vector) with a DMA per quarter, shortening the final
                    # compute->store chain on the last tile
                    ot = out_pool.tile([128, H], BF16, tag="ot", name=f"ot{t}")
                    for qtr in range(4):
                        cs = slice(qtr * 256, (qtr + 1) * 256)
                        if qtr < 2:
                            nc.scalar.activation(
                                ot[:, cs],
                                po[0][:, qtr * 256 : (qtr + 1) * 256],
                                mybir.ActivationFunctionType.Copy,
                                scale=rl,
                            )
                        else:
                            nc.vector.tensor_scalar_mul(
                                ot[:, cs], po[1][:, (qtr - 2) * 256 : (qtr - 1) * 256], rl
                            )
                        nc.sync.dma_start(out=out[t, :, cs], in_=ot[:, cs])

                # start with qs=0 tiles (their qt drains complete while the
                # qs=1 Q-projection matmuls still run); bury the small
                # classes mid-stream so every softmax hides under a
                # substantial scores/PV block
                order = [2, 3, 6, 7, 4, 5, 0, 1]
                pend = None
                for t in order:
                    ctx = emit_scores(t)
                    if pend is not None:
                        emit_pv(pend)
                    pend = ctx
                emit_pv(pend)

            psum_outer.__exit__(None, None, None)

    return nc


def kernel(inputs, Wq, bq, Wk, bk, Wv, bv):
    global LAST_RESULT
    inputs = np.ascontiguousarray(inputs, dtype=np.float32)
    scale = 1.0 / np.sqrt(np.float32(E))

    wqT = np.ascontiguousarray(Wq.T.astype(np.float32) * scale).astype(BF16_NP)
    wkT = np.ascontiguousarray(Wk.T.astype(np.float32)).astype(BF16_NP)
    wvT = np.ascontiguousarray(Wv.T.astype(np.float32)).astype(BF16_NP)
    bqs = (bq.astype(np.float32) * scale).copy()
    bk32 = np.ascontiguousarray(bk, dtype=np.float32)
    bv32 = np.ascontiguousarray(bv, dtype=np.float32)
    ident = np.eye(128, dtype=np.float32).astype(BF16_NP)

    jj = np.arange(512)
    in_maps = []
    for c in range(NCORES):
        b = c // 2
        h = c % 2
        xT = inputs[b].T  # [E, S]
        # permute key columns: within each 512 block, this core's two q-tiles
        # first, then the peer's two
        cols = np.empty(S, dtype=np.int64)
        for cb in range(4):
            for pos in range(4):
                g = 4 * cb + (pos + 2 * h) % 4
                cols[cb * 512 + pos * 128 : cb * 512 + (pos + 1) * 128] = np.arange(
                    g * 128, (g + 1) * 128
                )
        xP = np.ascontiguousarray(xT[:, cols]).astype(BF16_NP)
        mask = np.empty((NT, 128, 512), dtype=np.float32)
        for t in range(NT):
            cb = t // 2
            r = 4 * cb + 2 * h + (t % 2)
            jg = (4 * cb + (jj // 128 + 2 * h) % 4) * 128 + (jj % 128)
            ig = r * 128 + np.arange(128)[:, None]
            mask[t] = np.where(jg[None, :] <= ig, 0.0, -1e30).astype(np.float32)
        in_maps.append(
            {
                "xT": xP,
                "wqT": wqT,
                "wkT": wkT,
                "wvT": wvT,
                "bqs": bqs,
                "bk": bk32,
                "bv": bv32,
                "masks": mask,
                "ident": ident,
            }
        )

    nc = _build_program()
    res = None
    last_err = None
    for attempt in range(3):
        try:
            res = run_bass_kernel_spmd(nc, in_maps, list(range(NCORES)))
            break
        except Exception as e:  # transient NRT device wedge; retry
            last_err = e
            import time as _time

            _time.sleep(2.0)
    if res is None:
        raise last_err
    LAST_RESULT = res

    out = np.empty((B, S, H), dtype=np.float32)
    for c in range(NCORES):
        b = c // 2
        o = res.results[c]["out"]  # [NT, 128, H] bf16
        for t in range(NT):
            r = _global_tile(c, t)
            out[b, r * 128 : (r + 1) * 128, :] = np.asarray(o[t], dtype=np.float32)
    return out


# revision 39
# speedup vs baseline: 1.0204x; 1.0204x over previous
import os
import sys

sys.path.insert(0, "/opt/trn_rl_repo")

import numpy as np
import ml_dtypes
import bass_rust
from concourse import bass, mybir
from concourse.tile import TileContext
from concourse.vector_clock import ScopedClock
from concourse.bass_utils import run_bass_kernel_spmd

B, S, E, H = 4, 2048, 1024, 1024
NCORES = 8
NT = 8  # q-tiles per core, 128 rows each
EC = E // 128
HC = H // 128
KC = S // 128
F32 = mybir.dt.float32
BF16 = mybir.dt.bfloat16
BF16_NP = ml_dtypes.bfloat16

# Results of the last run_bass_kernel_spmd call (for test harness inspection).
LAST_RESULT = None


def _global_tile(core: int, t: int) -> int:
    """Local q-tile t on core -> global 128-row tile index r in [0, 16).

    Tiles are grouped in four causal classes (nb = t//2 + 1 key-blocks of
    512); each core takes two tiles per class so instruction streams are
    identical across cores.
    """
    half = core % 2
    return 4 * (t // 2) + 2 * half + (t % 2)


class PatchedTileContext(TileContext):
    """TileContext whose tail drain carries at most one sem wait.

    The walrus codegen in this container rejects a Drain with more than one
    sync wait ("Too many sync wait commands"); split the global-clock waits
    across a chain of drains on the same engine instead.
    """

    def _drain_and_barrier(self, tick_clock, wait_clock):
        drain_inst = self.nc.sync.drain()
        wait_clock.add_sem_waits(
            drain_inst.ins, ScopedClock({None: tick_clock.global_clock})
        )
        mi = drain_inst.ins
        waits = list(mi.sync_info.on_wait)
        ups = list(mi.sync_info.on_update)
        if len(waits) > 1:
            mi.sync_info = bass_rust.SyncInfo(on_wait=waits[:1], on_update=[])
            for i, w in enumerate(waits[1:]):
                d2 = self.nc.sync.drain()
                last = i == len(waits) - 2
                d2.ins.sync_info = bass_rust.SyncInfo(
                    on_wait=[w], on_update=ups if last else []
                )
        self.nc.all_engine_barrier()
        assert self.sems is not None
        popped = self.nc._tile_sem_poison_stack.pop()
        assert popped is self._sem_poison
        # Device-side semaphore/DMA-ring clears + the second barrier are only
        # needed if the NEFF executes again in the same NRT session; this
        # kernel runs once per launch, so skip ~8us of teardown. Host-side
        # bookkeeping (sem free-list) is irrelevant at the outermost tile.


def _split_multi_waits(json_bytes):
    """Rewrite BIR so no instruction carries more than one sync wait.

    The walrus build in this container rejects instructions with multiple
    sync waits ("Too many sync wait commands"). Engines execute in order, so
    hoisting the extra waits onto NoOp instructions inserted immediately
    before the original instruction is semantically equivalent.
    """
    import json as _json

    d = _json.loads(json_bytes)
    ctr = 0
    for f in d.get("functions", []):
        for blk in f.get("blocks", []):
            insts = blk.get("instructions", [])
            out = []
            for inst in insts:
                si = inst.get("sync_info") or {}
                ow = si.get("on_wait") or []
                if len(ow) > 1:
                    for w in ow[:-1]:
                        out.append(
                            {
                                "debug": inst.get("debug", 0),
                                "engine": inst["engine"],
                                "ins": [],
                                "name": f"wsplit_{ctr}",
                                "opcode": "NoOp",
                                "outs": [],
                                "sync_info": {"on_update": [], "on_wait": [w]},
                            }
                        )
                        ctr += 1
                    si = dict(si)
                    si["on_wait"] = [ow[-1]]
                    inst = dict(inst)
                    inst["sync_info"] = si
                out.append(inst)
            blk["instructions"] = out
    return _json.dumps(d).encode()


def _build_program():
    nc = bass.Bass("TRN2", target_bir_lowering=False, debug=False, num_devices=NCORES)
    orig_to_json_bytes = nc.to_json_bytes
    nc.to_json_bytes = lambda: _split_multi_waits(orig_to_json_bytes())

    # xT columns are host-permuted per core: within each 512-key block the
    # core's own two q-tiles come first, so q-tile t sits at fixed columns
    # (t//2)*512 + (t%2)*128 on EVERY core (uniform SPMD program).
    xT = nc.dram_tensor("xT", [E, S], BF16, kind="ExternalInput")
    wqT = nc.dram_tensor("wqT", [E, H], BF16, kind="ExternalInput")
    wkT = nc.dram_tensor("wkT", [E, H], BF16, kind="ExternalInput")
    wvT = nc.dram_tensor("wvT", [E, H], BF16, kind="ExternalInput")
    bqs = nc.dram_tensor("bqs", [H], F32, kind="ExternalInput")
    bk = nc.dram_tensor("bk", [H], F32, kind="ExternalInput")
    bv = nc.dram_tensor("bv", [H], F32, kind="ExternalInput")
    masks = nc.dram_tensor("masks", [NT, 128, 512], F32, kind="ExternalInput")
    ident = nc.dram_tensor("ident", [128, 128], BF16, kind="ExternalInput")
    out = nc.dram_tensor("out", [NT, 128, H], BF16, kind="ExternalOutput")

    with PatchedTileContext(nc) as tc:
        with (
            tc.tile_pool(name="const", bufs=1) as const_pool,
            tc.tile_pool(name="xp", bufs=1) as x_pool,
            tc.tile_pool(name="ktp", bufs=1) as kt_pool,
            tc.tile_pool(name="vp", bufs=1) as v_pool,
            tc.tile_pool(name="qtp", bufs=1) as qt_pool,
        ):
            cst = const_pool.tile([128, H + 2 * HC], F32, tag="cst")
            bv_bc = cst[:, 0:H]
            bq_t = cst[:, H : H + HC]
            bk_t = cst[:, H + HC : H + 2 * HC]
            idc = const_pool.tile([128, 128], BF16, tag="idc")

            # resident tensors (bf16): x for all three projections, K^T and V
            # for attention, Q^T per-tile slices
            x = x_pool.tile([128, EC, S], BF16, tag="x")
            kt = [
                kt_pool.tile([128, S], BF16, tag=f"kt{c}", name=f"kt{c}")
                for c in range(HC)
            ]
            v = [
                v_pool.tile([128, H], BF16, tag=f"v{k}", name=f"v{k}")
                for k in range(KC)
            ]
            # [128, hc, q-col]: q-tile t occupies columns t*128..t*128+128,
            # so each Q-projection drain writes a contiguous 512-col run
            qt = qt_pool.tile([128, HC, NT * 128], BF16, tag="qt")

            with (
                tc.tile_pool(name="wA", bufs=1) as wA_pool,
                tc.tile_pool(name="wB", bufs=1) as wB_pool,
            ):
                wk_sb = [
                    wA_pool.tile([128, H], BF16, tag=f"wA{e}", name=f"wk{e}")
                    for e in range(EC)
                ]
                wv_sb = [
                    wB_pool.tile([128, H], BF16, tag=f"wB{e}", name=f"wv{e}")
                    for e in range(EC)
                ]
                # interleave the first x columns with wk so the e=0 matmul can
                # issue early; stream the rest of x, then prefetch wv
                # spread the startup streams across four queue engines
                # (~140GB/s per queue) in consumption order so kb0 is never
                # starved; wq/wv carry slot-reuse waits so they must stay on
                # gpsimd (a waiting DMA would block a compute engine's queue)
                # stripe the startup streams round-robin across the three
                # DMA queue engines in consumption order: x-kb0 + first wk
                # half feed the split kb0 passes, then the rest of wk/x
                dma_engs = [nc.sync, nc.scalar, nc.gpsimd]
                rr = [0]

                def stripe(out_ap, in_ap):
                    dma_engs[rr[0] % 3].dma_start(out=out_ap, in_=in_ap)
                    rr[0] += 1

                for e in range(EC):
                    stripe(x[:, e, 0:512], xT[e * 128 : (e + 1) * 128, 0:512])
                    stripe(wk_sb[e][:, 0:512], wkT[e * 128 : (e + 1) * 128, 0:512])
                for e in range(EC):
                    stripe(wk_sb[e][:, 512:H], wkT[e * 128 : (e + 1) * 128, 512:H])
                for e in range(EC):
                    stripe(x[:, e, 512:1024], xT[e * 128 : (e + 1) * 128, 512:1024])
                nc.gpsimd.dma_start(
                    out=bq_t, in_=bqs[:].rearrange("(c p) -> p c", p=128)
                )
                nc.gpsimd.dma_start(
                    out=bk_t, in_=bk[:].rearrange("(c p) -> p c", p=128)
                )
                for e in range(EC):
                    xq_eng = nc.sync if e % 2 == 0 else nc.scalar
                    xq_eng.dma_start(
                        out=x[:, e, 1024:S], in_=xT[e * 128 : (e + 1) * 128, 1024:S]
                    )
                nc.gpsimd.dma_start(out=bv_bc, in_=bv[:].partition_broadcast(128))
                nc.sync.dma_start(out=idc, in_=ident[:, :])

                with tc.tile_pool(name="pp", bufs=8, space="PSUM") as pp_pool:
                    # ---- K projection: kt[hc] = (Wk x)^T + bk ----
                    # kb0 is split into two half-hc passes so the startup
                    # working set is 2MB (x-kb0 + wk half) instead of 3MB
                    for kb in range(4):
                        hc_groups = [[0, 1, 2, 3], [4, 5, 6, 7]] if kb == 0 else [
                            list(range(HC))
                        ]
                        for hcg in hc_groups:
                            ps = [
                                pp_pool.tile(
                                    [128, 512], F32, tag="pp", name=f"psk{kb}_{hc}"
                                )
                                for hc in hcg
                            ]
                            for e in range(EC):
                                for i, hc in enumerate(hcg):
                                    nc.tensor.matmul(
                                        ps[i],
                                        lhsT=wk_sb[e][:, hc * 128 : (hc + 1) * 128],
                                        rhs=x[:, e, kb * 512 : (kb + 1) * 512],
                                        start=(e == 0),
                                        stop=(e == EC - 1),
                                    )
                            for i, hc in enumerate(hcg):
                                nc.vector.tensor_scalar_add(
                                    kt[hc][:, kb * 512 : (kb + 1) * 512],
                                    ps[i],
                                    bk_t[:, hc : hc + 1],
                                )

                    # wq reuses the wk slots (tag reuse -> anti-dependency;
                    # with e-outer kb3 the per-e waits release early)
                    wq_sb = [
                        wA_pool.tile([128, H], BF16, tag=f"wA{e}", name=f"wq{e}")
                        for e in range(EC)
                    ]
                    for e in range(EC):
                        nc.gpsimd.dma_start(
                            out=wq_sb[e], in_=wqT[e * 128 : (e + 1) * 128, :]
                        )
                    for e in range(EC):
                        nc.gpsimd.dma_start(
                            out=wv_sb[e], in_=wvT[e * 128 : (e + 1) * 128, :]
                        )

                    # ---- Q projection (before V so the attention phase's
                    # PSUM handoff waits on V's cheap drains, not Q's):
                    # reads the core's q columns straight out of resident x
                    # (fixed offsets thanks to the host block permutation) ----
                    for qs in range(2):
                        ps = [
                            pp_pool.tile(
                                [128, 512], F32, tag="pp", name=f"psq{qs}_{hc}"
                            )
                            for hc in range(HC)
                        ]
                        for e in range(EC):
                            xr = x[:, e, :].rearrange(
                                "p (a b c) -> p a b c", a=4, b=4, c=128
                            )
                            rhs = xr[:, 2 * qs : 2 * qs + 2, 0:2, :]
                            for hc in range(HC):
                                nc.tensor.matmul(
                                    ps[hc],
                                    lhsT=wq_sb[e][:, hc * 128 : (hc + 1) * 128],
                                    rhs=rhs,
                                    start=(e == 0),
                                    stop=(e == EC - 1),
                                )
                        for hc in range(HC):
                            nc.vector.tensor_scalar_add(
                                qt[:, hc, qs * 512 : (qs + 1) * 512],
                                ps[hc],
                                bq_t[:, hc : hc + 1],
                            )

                    # ---- V projection: v[kc] = x_kc^T Wv + bv ----
                    for kc in range(KC):
                        ps2 = [
                            pp_pool.tile(
                                [128, 512], F32, tag="pp", name=f"psv{kc}_{hh}"
                            )
                            for hh in range(2)
                        ]
                        for e in range(EC):
                            for hh in range(2):
                                nc.tensor.matmul(
                                    ps2[hh],
                                    lhsT=x[:, e, kc * 128 : (kc + 1) * 128],
                                    rhs=wv_sb[e][:, hh * 512 : (hh + 1) * 512],
                                    start=(e == 0),
                                    stop=(e == EC - 1),
                                )
                        for hh in range(2):
                            nc.vector.tensor_add(
                                v[kc][:, hh * 512 : (hh + 1) * 512],
                                ps2[hh],
                                bv_bc[:, hh * 512 : (hh + 1) * 512],
                            )

            # ---- attention, classes descending, 1-deep software pipeline:
            # scores(t_next) overlaps softmax+PV(t_prev) ----
            with (
                tc.tile_pool(name="mskp", bufs=2) as msk_pool,
                tc.tile_pool(name="ssbp", bufs=2) as ssb_pool,
                tc.tile_pool(name="ptp", bufs=4) as pt_pool,
                tc.tile_pool(name="outp", bufs=2) as out_pool,
                tc.tile_pool(name="stat", bufs=4) as stat_pool,
                tc.tile_pool(name="sps", bufs=2, space="PSUM") as sps_pool,
                tc.tile_pool(name="ops", bufs=4, space="PSUM") as ops_pool,
                tc.tile_pool(name="tps", bufs=2, space="PSUM") as tps_pool,
            ):

                def emit_scores(t):
                    nb = t // 2 + 1
                    msk = msk_pool.tile([128, 512], F32, tag="msk", name=f"msk{t}")
                    nc.sync.dma_start(out=msk, in_=masks[t, :, :])
                    ssb = ssb_pool.tile([128, 4, 512], BF16, tag="ssb", name=f"ssb{t}")
                    mparts = stat_pool.tile([128, 4], F32, tag="mparts", name=f"mp{t}")
                    for kb in range(nb):
                        sp = sps_pool.tile([128, 512], F32, tag="sp", name=f"sp{t}_{kb}")
                        for hc in range(HC):
                            nc.tensor.matmul(
                                sp,
                                lhsT=qt[:, hc, t * 128 : (t + 1) * 128],
                                rhs=kt[hc][:, kb * 512 : (kb + 1) * 512],
                                start=(hc == 0),
                                stop=(hc == HC - 1),
                            )
                        if kb == nb - 1:
                            nc.vector.tensor_add(ssb[:, kb, :], sp, msk)
                        else:
                            nc.vector.tensor_copy(ssb[:, kb, :], sp)
                        nc.vector.reduce_max(
                            mparts[:, kb : kb + 1],
                            ssb[:, kb, :],
                            axis=mybir.AxisListType.X,
                        )
                    nm = stat_pool.tile([128, 1], F32, tag="nm", name=f"nm{t}")
                    nc.vector.reduce_max(
                        nm, mparts[:, :nb], axis=mybir.AxisListType.X, negate=True
                    )
                    lparts = stat_pool.tile([128, 4], F32, tag="lparts", name=f"lp{t}")
                    for kb in range(nb):
                        nc.scalar.activation(
                            ssb[:, kb, :],
                            ssb[:, kb, :],
                            mybir.ActivationFunctionType.Exp,
                            bias=nm,
                            accum_out=lparts[:, kb : kb + 1],
                        )
                    return (t, nb, ssb, lparts)

                def emit_pv(ctx):
                    t, nb, ssb, lparts = ctx
                    nkc = nb * 4
                    # for even tiles the own-odd key chunk of the final block
                    # is entirely causally masked (P exactly 0) -> skip it
                    skip_kc = 4 * (nb - 1) + 1 if t % 2 == 0 else -1
                    kcs = [kc for kc in range(nkc) if kc != skip_kc]
                    lsum = stat_pool.tile([128, 1], F32, tag="lsum", name=f"ls{t}")
                    nc.vector.reduce_sum(
                        lsum, lparts[:, :nb], axis=mybir.AxisListType.X
                    )
                    rl = stat_pool.tile([128, 1], F32, tag="rl", name=f"rl{t}")
                    nc.vector.reciprocal(rl, lsum)
                    po = [
                        ops_pool.tile([128, 512], F32, tag="po", name=f"po{t}_{hh}")
                        for hh in range(2)
                    ]
                    pts = {}

                    def do_T(kc):
                        tp = tps_pool.tile(
                            [128, 128], BF16, tag="tp", name=f"tp{t}_{kc}"
                        )
                        nc.tensor.transpose(
                            tp,
                            ssb[:, kc // 4, (kc % 4) * 128 : (kc % 4 + 1) * 128],
                            idc,
                        )
                        pt = pt_pool.tile([128, 128], BF16, tag="pt", name=f"pt{t}_{kc}")
                        nc.vector.tensor_copy(pt, tp)
                        pts[kc] = pt

                    for j in range(min(2, len(kcs))):
                        do_T(kcs[j])
                    for j, kc in enumerate(kcs):
                        if j + 2 < len(kcs):
                            do_T(kcs[j + 2])
                        for hh in range(2):
                            nc.tensor.matmul(
                                po[hh],
                                lhsT=pts[kc],
                                rhs=v[kc][:, hh * 512 : (hh + 1) * 512],
                                start=(j == 0),
                                stop=(j == len(kcs) - 1),
                            )
                    # normalize the two halves on different engines (scalar
                    # picked first so nothing queues ahead of the next tile's
                    # pt copies on vector); split the out-DMA so the first
                    # half's transfer starts while the second is computed
                    ot = out_pool.tile([128, H], BF16, tag="ot", name=f"ot{t}")
                    nc.scalar.activation(
                        ot[:, 0:512],
                        po[0],
                        mybir.ActivationFunctionType.Copy,
                        scale=rl,
                    )
                    nc.vector.tensor_scalar_mul(ot[:, 512:H], po[1], rl)
                    nc.sync.dma_start(out=out[t, :, 0:512], in_=ot[:, 0:512])
                    nc.sync.dma_start(out=out[t, :, 512:H], in_=ot[:, 512:H])

                # start with qs=0 tiles (their qt drains complete while the
                # qs=1 Q-projection matmuls still run); bury the small
                # classes mid-stream so every softmax hides under a
                # substantial scores/PV block
                order = [2, 3, 6, 7, 4, 5, 0, 1]
                pend = None
                for t in order:
                    ctx = emit_scores(t)
                    if pend is not None:
                        emit_pv(pend)
                    pend = ctx
                emit_pv(pend)

    return nc


def kernel(inputs, Wq, bq, Wk, bk, Wv, bv):
    global LAST_RESULT
    inputs = np.ascontiguousarray(inputs, dtype=np.float32)
    scale = 1.0 / np.sqrt(np.float32(E))

    wqT = np.ascontiguousarray(Wq.T.astype(np.float32) * scale).astype(BF16_NP)
    wkT = np.ascontiguousarray(Wk.T.astype(np.float32)).astype(BF16_NP)
    wvT = np.ascontiguousarray(Wv.T.astype(np.float32)).astype(BF16_NP)
    bqs = (bq.astype(np.float32) * scale).copy()
    bk32 = np.ascontiguousarray(bk, dtype=np.float32)
    bv32 = np.ascontiguousarray(bv, dtype=np.float32)
    ident = np.eye(128, dtype=np.float32).astype(BF16_NP)

    jj = np.arange(512)
    in_maps = []
    for c in range(NCORES):
        b = c // 2
        h = c % 2
        xT = inputs[b].T  # [E, S]
        # permute key columns: within each 512 block, this core's two q-tiles
        # first, then the peer's two
        cols = np.empty(S, dtype=np.int64)
        for cb in range(4):
            for pos in range(4):
                g = 4 * cb + (pos + 2 * h) % 4
                cols[cb * 512 + pos * 128 : cb * 512 + (pos + 1) * 128] = np.arange(
                    g * 128, (g + 1) * 128
                )
        xP = np.ascontiguousarray(xT[:, cols]).astype(BF16_NP)
        mask = np.empty((NT, 128, 512), dtype=np.float32)
        for t in range(NT):
            cb = t // 2
            r = 4 * cb + 2 * h + (t % 2)
            jg = (4 * cb + (jj // 128 + 2 * h) % 4) * 128 + (jj % 128)
            ig = r * 128 + np.arange(128)[:, None]
            mask[t] = np.where(jg[None, :] <= ig, 0.0, -1e30).astype(np.float32)
        in_maps.append(
            {
                "xT": xP,
                "wqT": wqT,
                "wkT": wkT,
                "wvT": wvT,
                "bqs": bqs,
                "bk": bk32,
                "bv": bv32,
                "masks": mask,
                "ident": ident,
            }
        )

    nc = _build_program()
    res = None
    last_err = None
    for attempt in range(3):
        try:
            res = run_bass_kernel_spmd(nc, in_maps, list(range(NCORES)))
            break
        except Exception as e:  # transient NRT device wedge; retry
            last_err = e
            import time as _time

            _time.sleep(2.0)
    if res is None:
        raise last_err
    LAST_RESULT = res

    out = np.empty((B, S, H), dtype=np.float32)
    for c in range(NCORES):
        b = c // 2
        o = res.results[c]["out"]  # [NT, 128, H] bf16
        for t in range(NT):
            r = _global_tile(c, t)
            out[b, r * 128 : (r + 1) * 128, :] = np.asarray(o[t], dtype=np.float32)
    return out
